# revision 3
# baseline (speedup 1.0000x reference)
"""Trainium2 Bass kernel for nn_Graph_to_Featuremaps_savemem.

Math: the reference computes, per batch b,
    scores[b,p,n] = (res @ nfr)[b,p] + (x @ nfh)[b,n]
    attn = softmax_n(scores);  out[b,p,c] = (attn @ (x @ W))[b,p,c]
Softmax over n is shift-invariant, so the (res @ nfr)[b,p] term cancels:
    attn[b,p,:] = softmax(x[b] @ nfh)   (independent of p)
    out[b,c,h,w] = relu(((softmax(x[b]@nfh) @ x[b]) @ W)[c])   broadcast over (h,w)
res_feature never affects the output. The kernel is therefore a tiny per-batch
compute (one 64-softmax + two small matmuls) followed by a 256 MB broadcast
write — pure HBM-write-bound, sharded batch-parallel over 8 cores (2 batches,
32 MB written per core).

The 32 MiB/core write runs at the 16-SDMA-engine fabric ceiling (~26.4
GB/s/engine), so the levers are time-to-first-output-packet and descriptor
size. Ramp: three parallel input DMAs with the critical piece (x^T|nfh)
first; a host-side transpose instead of a PE transpose; both per-batch
reductions batched into single matmuls via a block-diagonal mask (E2 =
e * mask, so X^T @ E2 gives both U_b columns and ones^T @ E2 both softmax
sums); a 1024-wide quick fill so block 1's first DMA issues right after the
~3 us compute chain. Bulk writes use broadcast-read DMAs whose SBUF-side
access pattern has a stride-0 middle dim ([[p,128],[0,3],[1,4096]])
re-reading the same fill window - deep queues from few instructions, 16 KiB
descriptors.
"""

import numpy as np

N_CORES = 8
B, NODES, HID, C, H, W = 16, 64, 128, 256, 128, 128
HWP = H * W  # 16384
B_LOC = B // N_CORES  # 2 batches per core

_NC_CACHE = {}


def build_nc():
    import concourse.bass as bass
    import concourse.bacc as bacc
    import concourse.mybir as mybir
    from concourse.tile import TileContext

    f32 = mybir.dt.float32
    Alu = mybir.AluOpType
    Act = mybir.ActivationFunctionType

    nc = bacc.Bacc(None, target_bir_lowering=False, debug=False)
    # ta = [x^T | nfh]; tb = [x | mask | ones]; tw = weight
    ta_d = nc.declare_dram_parameter("ta", [128, 129], f32, isOutput=False)
    tb_d = nc.declare_dram_parameter("tb", [128, 258], f32, isOutput=False)
    tw_d = nc.declare_dram_parameter("tw", [HID, C], f32, isOutput=False)
    out_d = nc.declare_dram_parameter("out", [B_LOC * C, HWP], f32, isOutput=True)

    def bcast_mid(ap, reps):
        # (P,F) AP -> (P,reps,F) AP re-reading the same F-wide window
        return type(ap)(ap.tensor, ap.offset, [list(ap.ap[0]), [0, reps], list(ap.ap[1])])

    with TileContext(nc) as tc:
        with (
            tc.tile_pool(name="singles", bufs=1) as singles,
            tc.tile_pool(name="fills", bufs=1) as fills,
            tc.tile_pool(name="psum", bufs=1, space="PSUM") as psum,
        ):
            ZERO = singles.tile([128, 2048], f32, tag="ZERO")
            nc.vector.memset(ZERO[:], 0.0)

            TA = singles.tile([128, 129], f32, tag="TA")
            nc.sync.dma_start(out=TA[:], in_=ta_d[:])
            TB = singles.tile([128, 258], f32, tag="TB")
            nc.sync.dma_start(out=TB[:], in_=tb_d[:])
            TW = singles.tile([HID, C], f32, tag="TW")
            nc.scalar.dma_start(out=TW[:], in_=tw_d[:])

            xt_ap = TA[:, 0:128]        # (hid, bn)
            nfh_ap = TA[:, 128:129]     # (hid, 1)
            x_ap = TB[:, 0:128]         # (bn, hid)
            mask_ap = TB[:, 128:130]    # (bn, 2) block-diagonal indicator
            onec_ap = TB[:, 130:131]    # (bn, 1) ones
            oner_ap = TB[0:1, 130:258]  # (1, 128) ones

            # s = x @ nfh  (bn, 1);  e = exp(s)  (unnormalized; scores are O(1))
            s_ps = psum.tile([B_LOC * NODES, 1], f32, tag="s")
            nc.tensor.matmul(s_ps[:], xt_ap, nfh_ap)
            e_sb = singles.tile([128, 1], f32, tag="e")
            nc.scalar.activation(e_sb[:], s_ps[:], Act.Exp)
            # E2[:, b] = e masked to batch b -> one matmul gives both U_b and sum_b
            E2 = singles.tile([128, B_LOC], f32, tag="E2")
            nc.vector.tensor_scalar(E2[:], mask_ap, e_sb[:], None, op0=Alu.mult)
            U2_ps = psum.tile([HID, B_LOC], f32, tag="U2")
            nc.tensor.matmul(U2_ps[:], x_ap, E2[:])
            S_ps = psum.tile([1, B_LOC], f32, tag="S")
            nc.tensor.matmul(S_ps[:], onec_ap, E2[:])
            r_sb = singles.tile([1, B_LOC], f32, tag="r")
            nc.vector.reciprocal(r_sb[:], S_ps[:])
            # RC[p, b] = 1/sum_b on every partition
            RC_ps = psum.tile([128, B_LOC], f32, tag="RC")
            nc.tensor.matmul(RC_ps[:], oner_ap, r_sb[:])
            RC = singles.tile([128, B_LOC], f32, tag="RC_sb")
            nc.vector.tensor_copy(RC[:], RC_ps[:])
            U2 = singles.tile([HID, B_LOC], f32, tag="U2_sb")
            nc.vector.tensor_copy(U2[:], U2_ps[:])

            # V_h = W_h^T @ U2 (c-major half h), VR = V * (1/sum) per column
            VRs = []
            for h in range(C // 128):
                V_ps = psum.tile([128, B_LOC], f32, tag=f"V{h}")
                nc.tensor.matmul(V_ps[:], TW[:, 128 * h : 128 * (h + 1)], U2[:])
                VR = singles.tile([128, B_LOC], f32, tag=f"VR{h}")
                nc.vector.tensor_mul(VR[:], V_ps[:], RC[:])
                VRs.append(VR)

            # blocks ordered by fill readiness: h=0 first (VR0 ready earlier)
            eng = [nc.sync, nc.scalar]
            k = 0
            for h in range(C // 128):
                for b in range(B_LOC):
                    r0 = b * C + h * 128
                    vcol = VRs[h][:, b : b + 1]
                    fill = fills.tile([128, 4096], f32, tag=f"fill{b}{h}")
                    if k == 0:
                        # quick start: 1024-wide fill -> first DMA ASAP
                        nc.vector.tensor_scalar(
                            fill[:, 0:1024], ZERO[:, 0:1024], vcol, 0.0,
                            op0=Alu.add, op1=Alu.max,
                        )
                        eng[0].dma_start(
                            out=out_d[r0 : r0 + 128, 0:1024], in_=fill[:, 0:1024]
                        )
                        eng[1].dma_start(
                            out=out_d[r0 : r0 + 128, 1024:4096],
                            in_=bcast_mid(fill[:, 0:1024], 3),
                        )
                        nc.vector.tensor_copy(fill[:, 1024:2048], fill[:, 0:1024])
                        nc.vector.tensor_copy(fill[:, 2048:4096], fill[:, 0:2048])
                        eng[0].dma_start(
                            out=out_d[r0 : r0 + 128, 4096:HWP],
                            in_=bcast_mid(fill[:, 0:4096], 3),
                        )
                    else:
                        nc.vector.tensor_scalar(
                            fill[:, 0:2048], ZERO[:], vcol, 0.0,
                            op0=Alu.add, op1=Alu.max,
                        )
                        nc.vector.tensor_copy(fill[:, 2048:4096], fill[:, 0:2048])
                        eng[k % 2].dma_start(
                            out=out_d[r0 : r0 + 128, 0:4096], in_=fill[:, 0:4096]
                        )
                        eng[(k + 1) % 2].dma_start(
                            out=out_d[r0 : r0 + 128, 4096:HWP],
                            in_=bcast_mid(fill[:, 0:4096], 3),
                        )
                    k += 1
    nc.finalize()
    return nc


def get_nc():
    if "nc" not in _NC_CACHE:
        _NC_CACHE["nc"] = build_nc()
    return _NC_CACHE["nc"]


def make_in_maps(input, node_fea_for_hidden, weight):
    x = np.asarray(input, np.float32)[0]  # (B, NODES, HID)
    nfh = np.asarray(node_fea_for_hidden, np.float32).reshape(HID, 1)
    w = np.ascontiguousarray(np.asarray(weight, np.float32))
    mask = np.zeros((B_LOC * NODES, B_LOC), np.float32)
    for b in range(B_LOC):
        mask[b * NODES : (b + 1) * NODES, b] = 1.0
    ones = np.ones((128, 128), np.float32)
    in_maps = []
    for i in range(N_CORES):
        xs = x[i * B_LOC : (i + 1) * B_LOC].reshape(B_LOC * NODES, HID)
        ta = np.concatenate([xs.T, nfh], axis=1, dtype=np.float32)
        tb = np.concatenate([xs, mask, ones], axis=1, dtype=np.float32)
        in_maps.append(
            {
                "ta": np.ascontiguousarray(ta),
                "tb": np.ascontiguousarray(tb),
                "tw": w,
            }
        )
    return in_maps


def run_spmd(in_maps, trace=False, **kw):
    from concourse.bass_utils import run_bass_kernel_spmd

    return run_bass_kernel_spmd(get_nc(), in_maps, list(range(N_CORES)), trace=trace, **kw)


def kernel(input, res_feature, node_fea_for_res, node_fea_for_hidden, weight):
    res = run_spmd(make_in_maps(input, node_fea_for_hidden, weight)).results
    out = np.concatenate(
        [r["out"].reshape(B_LOC, C, H, W) for r in res], axis=0
    )
    return out
